# revision 16
# baseline (speedup 1.0000x reference)
"""Expert-parallel MoE SwiGLU FFN kernel for 8 Trainium2 NeuronCores.

Problem: T=4096 tokens, DIM=1024, E=8 experts, INTER=1408, top-2 routing.
Reference computes all experts densely then gathers; we instead route on the
host (sort token-slots by expert), assign one expert per core, and each core
runs a SwiGLU FFN over only its routed tokens (padded to a common capacity so
all 8 cores execute the same SPMD program).

Device layout (per core, everything "transposed" with tokens on the free dim):
  xt  [8,128,C]  bf16   x_gathered.T tiled over DIM      (k-tile, partition, token)
  w1t [8,128,1408] bf16 w1[e].T tiled over DIM
  w3t [8,128,1408] bf16
  w2t [11,128,1024] bf16 w2[e].T tiled over INTER
  yt  [8,128,C]  f32    y.T tiled over DIM (output)

Compute per core:
  h1.T = w1 @ x.T   (accumulate over 8 DIM k-tiles)     -> PSUM [128, n]
  h3.T = w3 @ x.T
  g.T  = silu(h1.T) * h3.T                              -> SBUF bf16
  y.T  = w2 @ g.T   (accumulate over 11 INTER m-tiles)  -> PSUM -> SBUF f32 -> HBM
"""

import numpy as np
import ml_dtypes

T, DIM, E, INTER, TOPK = 4096, 1024, 8, 1408, 2
NCORES = 8
P = 128
KT = DIM // P    # 8 k-tiles over DIM
MT = INTER // P  # 11 m-tiles over INTER

TRACE = False  # test.py sets this to capture an NTFF profile
LAST_RESULTS = None  # BassKernelResults of the last run (for test.py)

_NC_CACHE = {}


def _chunks_for(C):
    # Split C into equal-ish chunks of at most 512 (PSUM bank = 512 fp32),
    # multiples of 16, avoiding a tiny LDWEIGHTS-bound tail chunk.
    nch = -(-C // 512)
    base = C // nch
    out = []
    rem = C
    for i in range(nch, 0, -1):
        n = min(512, -(-rem // i))
        n = -(-n // 16) * 16 if i > 1 else rem  # keep multiples of 16
        n = min(n, 512, rem)
        out.append(n)
        rem -= n
    assert sum(out) == C and all(0 < n <= 512 for n in out), out
    return out


def _build_nc(C):
    import concourse.mybir as mybir
    import concourse.tile as tile
    from concourse import bacc
    from concourse.tile_rust import add_dep_helper

    dt = mybir.dt
    AF = mybir.ActivationFunctionType
    chunks = _chunks_for(C)

    nc = bacc.Bacc("TRN2", target_bir_lowering=False, debug=False)
    # x is stored chunk-major: one contiguous [P, KT, n] block per chunk so a
    # single full-rate DMA delivers each chunk. w1/w3 are m-column-major
    # ([MT, P, KT, 128]) so weight DMAs land in phase-A consumption order.
    xts = [
        nc.dram_tensor(f"xt{j}", [P, KT, n], dt.bfloat16, kind="ExternalInput")
        for j, n in enumerate(chunks)
    ]
    w1t = nc.dram_tensor("w1t", [MT, P, KT, P], dt.bfloat16, kind="ExternalInput")
    w3t = nc.dram_tensor("w3t", [MT, P, KT, P], dt.bfloat16, kind="ExternalInput")
    w2t = nc.dram_tensor("w2t", [MT, P, DIM], dt.bfloat16, kind="ExternalInput")
    yt = nc.dram_tensor("yt", [KT, P, C], dt.float32, kind="ExternalOutput")

    with tile.TileContext(nc) as tc:
        with (
            tc.tile_pool(name="persist", bufs=1) as wpool,
            tc.tile_pool(name="gbuf", bufs=2) as gpool,
            tc.tile_pool(name="ybuf", bufs=3) as ypool,
            tc.tile_pool(name="silbuf", bufs=3) as spool,
            tc.tile_pool(name="psA", bufs=3, space="PSUM") as psA,
            tc.tile_pool(name="psB", bufs=2, space="PSUM") as psB,
        ):
            # SBUF layouts mirror the DRAM layouts so every DMA is contiguous
            # on both sides: per-chunk x tiles, m-major w1/w3.
            xss = [wpool.tile([P, KT, n], dt.bfloat16, name=f"xs{j}")
                   for j, n in enumerate(chunks)]
            w1s = wpool.tile([P, MT, KT * P], dt.bfloat16)
            w3s = wpool.tile([P, MT, KT * P], dt.bfloat16)
            w2s = wpool.tile([P, MT, DIM], dt.bfloat16)
            n0 = chunks[0]
            # chunk-0 x first, then weights in phase-A consumption order
            # (m-column pairs); remaining x chunks interleaved early enough.
            # Critical prefetch (full bandwidth): chunk-0 x and the first two
            # weight column-pairs. Everything else is deferred behind the
            # first real matmul via explicit deps, so it doesn't steal DMA
            # bandwidth from the startup-critical transfers.
            deferred = []
            nc.sync.dma_start(xss[0][:], xts[0][:])
            for m in range(MT):
                d1 = nc.sync.dma_start(w1s[:, m, :], w1t[m])
                d3 = nc.sync.dma_start(w3s[:, m, :], w3t[m])
                if m >= 2:
                    deferred += [d1, d3]
            for j in range(1, len(chunks)):
                deferred.append(nc.sync.dma_start(xss[j][:], xts[j][:]))
            for m in range(MT):
                deferred.append(nc.sync.dma_start(w2s[:, m, :], w2t[m]))

            # PE warm-up: dummy matmuls on the first-arrived x tile keep the
            # tensor engine busy during the DMA fill so HAM unthrottles to
            # 2.4 GHz before the real matmuls start.
            warm_ps = psB.tile([P, 512], dt.float32, name="py")
            for _ in range(24):
                nc.tensor.matmul(
                    warm_ps[:, :n0],
                    xss[0][:, 0, :P],
                    xss[0][:, 0, :],
                    start=True,
                    stop=True,
                )

            c0 = 0
            first_mm = None
            for j, n in enumerate(chunks):
                xsj = xss[j]
                gs = gpool.tile([P, MT, n], dt.bfloat16, name="gs")
                for m in range(MT):
                    p1 = psA.tile([P, n], dt.float32, name="p1")
                    p3 = psA.tile([P, n], dt.float32, name="p3")
                    for k in range(KT):
                        mm = nc.tensor.matmul(
                            p1[:],
                            w1s[:, m, k * P:(k + 1) * P],
                            xsj[:, k, :],
                            start=(k == 0),
                            stop=(k == KT - 1),
                        )
                        if first_mm is None:
                            first_mm = mm
                            for d in deferred:
                                add_dep_helper(
                                    first_mm.ins, d.ins, sync=True,
                                    reason="defer bulk DMA until startup-critical matmul",
                                )
                    for k in range(KT):
                        nc.tensor.matmul(
                            p3[:],
                            w3s[:, m, k * P:(k + 1) * P],
                            xsj[:, k, :],
                            start=(k == 0),
                            stop=(k == KT - 1),
                        )
                    sil = spool.tile([P, n], dt.bfloat16, name="sil")
                    nc.scalar.activation(sil[:], p1[:], AF.Silu)
                    nc.vector.tensor_mul(gs[:, m, :], sil[:], p3[:])
                for i in range(KT):
                    py = psB.tile([P, n], dt.float32, name="py")
                    for m in range(MT):
                        nc.tensor.matmul(
                            py[:],
                            w2s[:, m, i * P:(i + 1) * P],
                            gs[:, m, :],
                            start=(m == 0),
                            stop=(m == MT - 1),
                        )
                    ys = ypool.tile([P, n], dt.float32, name="ys")
                    nc.vector.tensor_copy(ys[:], py[:])
                    nc.sync.dma_start(yt[i, :, c0:c0 + n], ys[:])
                c0 += n

    nc.compile()
    return nc


def _get_nc(C):
    if C not in _NC_CACHE:
        _NC_CACHE[C] = _build_nc(C)
    return _NC_CACHE[C]


def kernel(x, expert_indices, w1, w2, w3):
    global LAST_RESULTS
    from concourse import bass_utils

    x = np.asarray(x, dtype=np.float32)
    idx = np.asarray(expert_indices)
    out_dtype_idx = idx.dtype  # preserved implicitly; output is float32 anyway
    w1 = np.asarray(w1, dtype=np.float32)
    w2 = np.asarray(w2, dtype=np.float32)
    w3 = np.asarray(w3, dtype=np.float32)

    bf16 = ml_dtypes.bfloat16

    # --- host routing: stable-sort the (token, k) slots by expert id ---
    flat = idx.reshape(-1).astype(np.int64)  # slot s = t*TOPK + k -> expert
    order = np.argsort(flat, kind="stable")  # slots grouped by expert
    counts = np.bincount(flat, minlength=E)
    starts = np.zeros(E + 1, dtype=np.int64)
    np.cumsum(counts, out=starts[1:])
    cmax = int(counts.max())
    C = max(512, -(-cmax // 16) * 16)  # pad capacity to a multiple of 16

    nc = _get_nc(C)

    chunks = _chunks_for(C)
    bounds = np.cumsum([0] + chunks)
    xb = x.astype(bf16)
    in_maps = []
    for e in range(E):
        slots = order[starts[e]:starts[e + 1]]
        tokens = slots // TOPK
        xg = np.zeros((C, DIM), dtype=bf16)
        xg[: len(tokens)] = xb[tokens]
        # [C, DIM] -> [P, KT, C] (partition-major), then per-chunk blocks
        xpkc = xg.T.reshape(KT, P, C).transpose(1, 0, 2)
        im = {
            f"xt{j}": np.ascontiguousarray(xpkc[:, :, bounds[j]:bounds[j + 1]])
            for j in range(len(chunks))
        }
        # w1t[m, p, k, j] = w1[e][m*128+j, k*128+p]
        im["w1t"] = np.ascontiguousarray(
            w1[e].astype(bf16).reshape(MT, P, KT, P).transpose(0, 3, 2, 1)
        )
        im["w3t"] = np.ascontiguousarray(
            w3[e].astype(bf16).reshape(MT, P, KT, P).transpose(0, 3, 2, 1)
        )
        im["w2t"] = np.ascontiguousarray(w2[e].T.astype(bf16)).reshape(MT, P, DIM)
        in_maps.append(im)

    res = bass_utils.run_bass_kernel_spmd(
        nc, in_maps, core_ids=list(range(NCORES)), trace=TRACE
    )
    LAST_RESULTS = res

    out = np.empty((T * TOPK, DIM), dtype=np.float32)
    for e in range(E):
        slots = order[starts[e]:starts[e + 1]]
        yt = res.results[e]["yt"]  # [KT, P, C] f32
        y = yt.reshape(DIM, C)  # y.T
        out[slots] = y[:, : len(slots)].T
    return out.reshape(T, TOPK, DIM)


# revision 19
# speedup vs baseline: 1.1364x; 1.1364x over previous
"""Expert-parallel MoE SwiGLU FFN kernel for 8 Trainium2 NeuronCores.

Problem: T=4096 tokens, DIM=1024, E=8 experts, INTER=1408, top-2 routing.
Reference computes all experts densely then gathers; we instead route on the
host (sort token-slots by expert), assign one expert per core, and each core
runs a SwiGLU FFN over only its routed tokens (padded to a common capacity so
all 8 cores execute the same SPMD program).

Device layout (per core, everything "transposed" with tokens on the free dim):
  xt  [8,128,C]  bf16   x_gathered.T tiled over DIM      (k-tile, partition, token)
  w1t [8,128,1408] bf16 w1[e].T tiled over DIM
  w3t [8,128,1408] bf16
  w2t [11,128,1024] bf16 w2[e].T tiled over INTER
  yt  [8,128,C]  f32    y.T tiled over DIM (output)

Compute per core:
  h1.T = w1 @ x.T   (accumulate over 8 DIM k-tiles)     -> PSUM [128, n]
  h3.T = w3 @ x.T
  g.T  = silu(h1.T) * h3.T                              -> SBUF bf16
  y.T  = w2 @ g.T   (accumulate over 11 INTER m-tiles)  -> PSUM -> SBUF f32 -> HBM
"""

import numpy as np
import ml_dtypes

T, DIM, E, INTER, TOPK = 4096, 1024, 8, 1408, 2
NCORES = 8
P = 128
KT = DIM // P    # 8 k-tiles over DIM
MT = INTER // P  # 11 m-tiles over INTER

TRACE = False  # test.py sets this to capture an NTFF profile
LAST_RESULTS = None  # BassKernelResults of the last run (for test.py)

_NC_CACHE = {}


def _chunks_for(C):
    # Split C into equal-ish chunks of at most 512 (PSUM bank = 512 fp32),
    # multiples of 16, avoiding a tiny LDWEIGHTS-bound tail chunk.
    nch = -(-C // 512)
    base = C // nch
    out = []
    rem = C
    for i in range(nch, 0, -1):
        n = min(512, -(-rem // i))
        n = -(-n // 16) * 16 if i > 1 else rem  # keep multiples of 16
        n = min(n, 512, rem)
        out.append(n)
        rem -= n
    assert sum(out) == C and all(0 < n <= 512 for n in out), out
    return out


def _build_nc(C):
    import concourse.mybir as mybir
    import concourse.tile as tile
    from concourse import bacc

    dt = mybir.dt
    AF = mybir.ActivationFunctionType
    chunks = _chunks_for(C)

    nc = bacc.Bacc("TRN2", target_bir_lowering=False, debug=False)
    # x is stored chunk-major: one contiguous [P, KT, n] block per chunk so a
    # single full-rate DMA delivers each chunk. w1/w3 are m-column-major
    # ([MT, P, KT, 128]) so weight DMAs land in phase-A consumption order.
    xts = [
        nc.dram_tensor(f"xt{j}", [P, KT, n], dt.bfloat16, kind="ExternalInput")
        for j, n in enumerate(chunks)
    ]
    w1t = nc.dram_tensor("w1t", [MT, P, KT, P], dt.bfloat16, kind="ExternalInput")
    w3t = nc.dram_tensor("w3t", [MT, P, KT, P], dt.bfloat16, kind="ExternalInput")
    w2t = nc.dram_tensor("w2t", [MT, P, DIM], dt.bfloat16, kind="ExternalInput")
    yt = nc.dram_tensor("yt", [KT, P, C], dt.float32, kind="ExternalOutput")

    with tile.TileContext(nc) as tc:
        with (
            tc.tile_pool(name="persist", bufs=1) as wpool,
            tc.tile_pool(name="gbuf", bufs=2) as gpool,
            tc.tile_pool(name="ybuf", bufs=3) as ypool,
            tc.tile_pool(name="silbuf", bufs=3) as spool,
            tc.tile_pool(name="psA", bufs=3, space="PSUM") as psA,
            tc.tile_pool(name="psB", bufs=2, space="PSUM") as psB,
        ):
            # SBUF layouts mirror the DRAM layouts so every DMA is contiguous
            # on both sides: per-chunk x tiles, m-major w1/w3.
            xss = [wpool.tile([P, KT, n], dt.bfloat16, name=f"xs{j}")
                   for j, n in enumerate(chunks)]
            w1s = wpool.tile([P, MT, KT * P], dt.bfloat16)
            w3s = wpool.tile([P, MT, KT * P], dt.bfloat16)
            w2s = wpool.tile([P, MT, DIM], dt.bfloat16)
            n0 = chunks[0]
            # chunk-0 x first, then weights in phase-A consumption order
            # (m-column pairs); remaining x chunks interleaved early enough.
            nc.sync.dma_start(xss[0][:], xts[0][:])
            nxt = 1
            for m in range(MT):
                nc.sync.dma_start(w1s[:, m, :], w1t[m])
                nc.sync.dma_start(w3s[:, m, :], w3t[m])
                if m in (2, 4) and nxt < len(chunks):
                    nc.sync.dma_start(xss[nxt][:], xts[nxt][:])
                    nxt += 1
            while nxt < len(chunks):
                nc.sync.dma_start(xss[nxt][:], xts[nxt][:])
                nxt += 1
            for m in range(MT):
                nc.sync.dma_start(w2s[:, m, :], w2t[m])

            # PE warm-up: dummy matmuls on the first-arrived x tile keep the
            # tensor engine busy during the DMA fill so HAM unthrottles to
            # 2.4 GHz before the real matmuls start.
            warm_ps = psB.tile([P, 512], dt.float32, name="py")
            for _ in range(24):
                nc.tensor.matmul(
                    warm_ps[:, :n0],
                    xss[0][:, 0, :P],
                    xss[0][:, 0, :],
                    start=True,
                    stop=True,
                )

            c0 = 0
            for j, n in enumerate(chunks):
                xsj = xss[j]
                gs = gpool.tile([P, MT, n], dt.bfloat16, name="gs")
                for m in range(MT):
                    p1 = psA.tile([P, n], dt.float32, name="p1")
                    p3 = psA.tile([P, n], dt.float32, name="p3")
                    for k in range(KT):
                        nc.tensor.matmul(
                            p1[:],
                            w1s[:, m, k * P:(k + 1) * P],
                            xsj[:, k, :],
                            start=(k == 0),
                            stop=(k == KT - 1),
                        )
                    for k in range(KT):
                        nc.tensor.matmul(
                            p3[:],
                            w3s[:, m, k * P:(k + 1) * P],
                            xsj[:, k, :],
                            start=(k == 0),
                            stop=(k == KT - 1),
                        )
                    sil = spool.tile([P, n], dt.bfloat16, name="sil")
                    nc.scalar.activation(sil[:], p1[:], AF.Silu)
                    nc.vector.tensor_mul(gs[:, m, :], sil[:], p3[:])
                for i in range(KT):
                    py = psB.tile([P, n], dt.float32, name="py")
                    for m in range(MT):
                        nc.tensor.matmul(
                            py[:],
                            w2s[:, m, i * P:(i + 1) * P],
                            gs[:, m, :],
                            start=(m == 0),
                            stop=(m == MT - 1),
                        )
                    ys = ypool.tile([P, n], dt.float32, name="ys")
                    nc.vector.tensor_copy(ys[:], py[:])
                    nc.sync.dma_start(yt[i, :, c0:c0 + n], ys[:])
                c0 += n

    nc.compile()
    return nc


def _get_nc(C):
    if C not in _NC_CACHE:
        _NC_CACHE[C] = _build_nc(C)
    return _NC_CACHE[C]


def kernel(x, expert_indices, w1, w2, w3):
    global LAST_RESULTS
    from concourse import bass_utils

    x = np.asarray(x, dtype=np.float32)
    idx = np.asarray(expert_indices)
    out_dtype_idx = idx.dtype  # preserved implicitly; output is float32 anyway
    w1 = np.asarray(w1, dtype=np.float32)
    w2 = np.asarray(w2, dtype=np.float32)
    w3 = np.asarray(w3, dtype=np.float32)

    bf16 = ml_dtypes.bfloat16

    # --- host routing: stable-sort the (token, k) slots by expert id ---
    flat = idx.reshape(-1).astype(np.int64)  # slot s = t*TOPK + k -> expert
    order = np.argsort(flat, kind="stable")  # slots grouped by expert
    counts = np.bincount(flat, minlength=E)
    starts = np.zeros(E + 1, dtype=np.int64)
    np.cumsum(counts, out=starts[1:])
    cmax = int(counts.max())
    C = max(512, -(-cmax // 16) * 16)  # pad capacity to a multiple of 16

    nc = _get_nc(C)

    chunks = _chunks_for(C)
    bounds = np.cumsum([0] + chunks)
    xb = x.astype(bf16)
    in_maps = []
    for e in range(E):
        slots = order[starts[e]:starts[e + 1]]
        tokens = slots // TOPK
        xg = np.zeros((C, DIM), dtype=bf16)
        xg[: len(tokens)] = xb[tokens]
        # [C, DIM] -> [P, KT, C] (partition-major), then per-chunk blocks
        xpkc = xg.T.reshape(KT, P, C).transpose(1, 0, 2)
        im = {
            f"xt{j}": np.ascontiguousarray(xpkc[:, :, bounds[j]:bounds[j + 1]])
            for j in range(len(chunks))
        }
        # w1t[m, p, k, j] = w1[e][m*128+j, k*128+p]
        im["w1t"] = np.ascontiguousarray(
            w1[e].astype(bf16).reshape(MT, P, KT, P).transpose(0, 3, 2, 1)
        )
        im["w3t"] = np.ascontiguousarray(
            w3[e].astype(bf16).reshape(MT, P, KT, P).transpose(0, 3, 2, 1)
        )
        im["w2t"] = np.ascontiguousarray(w2[e].T.astype(bf16)).reshape(MT, P, DIM)
        in_maps.append(im)

    res = bass_utils.run_bass_kernel_spmd(
        nc, in_maps, core_ids=list(range(NCORES)), trace=TRACE
    )
    LAST_RESULTS = res

    out = np.empty((T * TOPK, DIM), dtype=np.float32)
    for e in range(E):
        slots = order[starts[e]:starts[e + 1]]
        yt = res.results[e]["yt"]  # [KT, P, C] f32
        y = yt.reshape(DIM, C)  # y.T
        out[slots] = y[:, : len(slots)].T
    return out.reshape(T, TOPK, DIM)


# revision 24
# speedup vs baseline: 1.1370x; 1.0006x over previous
"""Expert-parallel MoE SwiGLU FFN kernel for 8 Trainium2 NeuronCores.

Problem: T=4096 tokens, DIM=1024, E=8 experts, INTER=1408, top-2 routing.
Reference computes all experts densely then gathers; we instead route on the
host (sort token-slots by expert), assign one expert per core, and each core
runs a SwiGLU FFN over only its routed tokens (padded to a common capacity so
all 8 cores execute the same SPMD program).

Device layout (per core, everything "transposed" with tokens on the free dim):
  xt  [8,128,C]  bf16   x_gathered.T tiled over DIM      (k-tile, partition, token)
  w1t [8,128,1408] bf16 w1[e].T tiled over DIM
  w3t [8,128,1408] bf16
  w2t [11,128,1024] bf16 w2[e].T tiled over INTER
  yt  [8,128,C]  f32    y.T tiled over DIM (output)

Compute per core:
  h1.T = w1 @ x.T   (accumulate over 8 DIM k-tiles)     -> PSUM [128, n]
  h3.T = w3 @ x.T
  g.T  = silu(h1.T) * h3.T                              -> SBUF bf16
  y.T  = w2 @ g.T   (accumulate over 11 INTER m-tiles)  -> PSUM -> SBUF f32 -> HBM
"""

import numpy as np
import ml_dtypes

T, DIM, E, INTER, TOPK = 4096, 1024, 8, 1408, 2
NCORES = 8
P = 128
KT = DIM // P    # 8 k-tiles over DIM
MT = INTER // P  # 11 m-tiles over INTER

TRACE = False  # test.py sets this to capture an NTFF profile
LAST_RESULTS = None  # BassKernelResults of the last run (for test.py)

_NC_CACHE = {}


def _chunks_for(C):
    # Split C into equal-ish chunks of at most 512 (PSUM bank = 512 fp32),
    # multiples of 16, avoiding a tiny LDWEIGHTS-bound tail chunk.
    nch = -(-C // 512)
    base = C // nch
    out = []
    rem = C
    for i in range(nch, 0, -1):
        n = min(512, -(-rem // i))
        n = -(-n // 16) * 16 if i > 1 else rem  # keep multiples of 16
        n = min(n, 512, rem)
        out.append(n)
        rem -= n
    assert sum(out) == C and all(0 < n <= 512 for n in out), out
    return out


def _build_nc(C):
    import concourse.mybir as mybir
    import concourse.tile as tile
    from concourse import bacc

    dt = mybir.dt
    AF = mybir.ActivationFunctionType
    chunks = _chunks_for(C)

    nc = bacc.Bacc("TRN2", target_bir_lowering=False, debug=False)
    # x is stored chunk-major: one contiguous [P, KT, n] block per chunk so a
    # single full-rate DMA delivers each chunk. w1/w3 are m-column-major
    # ([MT, P, KT, 128]) so weight DMAs land in phase-A consumption order.
    xts = [
        nc.dram_tensor(f"xt{j}", [P, KT, n], dt.bfloat16, kind="ExternalInput")
        for j, n in enumerate(chunks)
    ]
    w1t = nc.dram_tensor("w1t", [MT, P, KT * P], dt.bfloat16, kind="ExternalInput")
    w3t = nc.dram_tensor("w3t", [MT, P, KT * P], dt.bfloat16, kind="ExternalInput")
    w2t = nc.dram_tensor("w2t", [MT, P, DIM], dt.bfloat16, kind="ExternalInput")
    yt = nc.dram_tensor("yt", [KT, P, C], dt.float32, kind="ExternalOutput")

    with tile.TileContext(nc) as tc:
        with (
            tc.tile_pool(name="persist", bufs=1) as wpool,
            tc.tile_pool(name="gbuf", bufs=2) as gpool,
            tc.tile_pool(name="ybuf", bufs=3) as ypool,
            tc.tile_pool(name="silbuf", bufs=3) as spool,
            tc.tile_pool(name="psA", bufs=3, space="PSUM") as psA,
            tc.tile_pool(name="psB", bufs=2, space="PSUM") as psB,
        ):
            # SBUF layouts mirror the DRAM layouts so every DMA is contiguous
            # on both sides: per-chunk x tiles, m-major w1/w3.
            xss = [wpool.tile([P, KT, n], dt.bfloat16, name=f"xs{j}")
                   for j, n in enumerate(chunks)]
            w1s = wpool.tile([P, MT, KT * P], dt.bfloat16)
            w3s = wpool.tile([P, MT, KT * P], dt.bfloat16)
            w2s = wpool.tile([P, MT, DIM], dt.bfloat16)
            n0 = chunks[0]
            # chunk-0 x first (split across two queues so it lands sooner),
            # then weights in phase-A consumption order (m-column pairs
            # individually for the first few, merged bulk transfers for the
            # rest to cut per-DMA trigger overhead on the Sync queue).
            nc.sync.dma_start(xss[0][:, :KT // 2, :], xts[0][:, :KT // 2, :])
            nc.sync.dma_start(xss[0][:, KT // 2:, :], xts[0][:, KT // 2:, :])
            for m in range(4):
                nc.sync.dma_start(w1s[:, m, :], w1t[m])
                nc.sync.dma_start(w3s[:, m, :], w3t[m])
            for j in range(1, len(chunks)):
                nc.sync.dma_start(xss[j][:], xts[j][:])
            nc.sync.dma_start(w1s[:, 4:8, :], w1t[4:8].rearrange("m p q -> p m q"))
            nc.sync.dma_start(w3s[:, 4:8, :], w3t[4:8].rearrange("m p q -> p m q"))
            nc.sync.dma_start(w1s[:, 8:, :], w1t[8:].rearrange("m p q -> p m q"))
            nc.sync.dma_start(w3s[:, 8:, :], w3t[8:].rearrange("m p q -> p m q"))
            nc.sync.dma_start(w2s[:, :6, :], w2t[:6].rearrange("m p q -> p m q"))
            nc.sync.dma_start(w2s[:, 6:, :], w2t[6:].rearrange("m p q -> p m q"))

            c0 = 0
            for j, n in enumerate(chunks):
                xsj = xss[j]
                gs = gpool.tile([P, MT, n], dt.bfloat16, name="gs")
                for m in range(MT):
                    p1 = psA.tile([P, n], dt.float32, name="p1")
                    p3 = psA.tile([P, n], dt.float32, name="p3")
                    for k in range(KT):
                        nc.tensor.matmul(
                            p1[:],
                            w1s[:, m, k * P:(k + 1) * P],
                            xsj[:, k, :],
                            start=(k == 0),
                            stop=(k == KT - 1),
                        )
                    for k in range(KT):
                        nc.tensor.matmul(
                            p3[:],
                            w3s[:, m, k * P:(k + 1) * P],
                            xsj[:, k, :],
                            start=(k == 0),
                            stop=(k == KT - 1),
                        )
                    sil = spool.tile([P, n], dt.bfloat16, name="sil")
                    nc.scalar.activation(sil[:], p1[:], AF.Silu)
                    nc.vector.tensor_mul(gs[:, m, :], sil[:], p3[:])
                for i in range(KT):
                    py = psB.tile([P, n], dt.float32, name="py")
                    for m in range(MT):
                        nc.tensor.matmul(
                            py[:],
                            w2s[:, m, i * P:(i + 1) * P],
                            gs[:, m, :],
                            start=(m == 0),
                            stop=(m == MT - 1),
                        )
                    ys = ypool.tile([P, n], dt.float32, name="ys")
                    nc.vector.tensor_copy(ys[:], py[:])
                    nc.gpsimd.dma_start(yt[i, :, c0:c0 + n], ys[:])
                c0 += n

    nc.compile()
    return nc


def _get_nc(C):
    if C not in _NC_CACHE:
        _NC_CACHE[C] = _build_nc(C)
    return _NC_CACHE[C]


def kernel(x, expert_indices, w1, w2, w3):
    global LAST_RESULTS
    from concourse import bass_utils

    x = np.asarray(x, dtype=np.float32)
    idx = np.asarray(expert_indices)
    out_dtype_idx = idx.dtype  # preserved implicitly; output is float32 anyway
    w1 = np.asarray(w1, dtype=np.float32)
    w2 = np.asarray(w2, dtype=np.float32)
    w3 = np.asarray(w3, dtype=np.float32)

    bf16 = ml_dtypes.bfloat16

    # --- host routing: stable-sort the (token, k) slots by expert id ---
    flat = idx.reshape(-1).astype(np.int64)  # slot s = t*TOPK + k -> expert
    order = np.argsort(flat, kind="stable")  # slots grouped by expert
    counts = np.bincount(flat, minlength=E)
    starts = np.zeros(E + 1, dtype=np.int64)
    np.cumsum(counts, out=starts[1:])
    cmax = int(counts.max())
    C = max(512, -(-cmax // 16) * 16)  # pad capacity to a multiple of 16

    nc = _get_nc(C)

    chunks = _chunks_for(C)
    bounds = np.cumsum([0] + chunks)
    xb = x.astype(bf16)
    in_maps = []
    for e in range(E):
        slots = order[starts[e]:starts[e + 1]]
        tokens = slots // TOPK
        xg = np.zeros((C, DIM), dtype=bf16)
        xg[: len(tokens)] = xb[tokens]
        # [C, DIM] -> [P, KT, C] (partition-major), then per-chunk blocks
        xpkc = xg.T.reshape(KT, P, C).transpose(1, 0, 2)
        im = {
            f"xt{j}": np.ascontiguousarray(xpkc[:, :, bounds[j]:bounds[j + 1]])
            for j in range(len(chunks))
        }
        # w1t[m, p, k*128+j] = w1[e][m*128+j, k*128+p]
        im["w1t"] = np.ascontiguousarray(
            w1[e].astype(bf16).reshape(MT, P, KT, P).transpose(0, 3, 2, 1)
        ).reshape(MT, P, KT * P)
        im["w3t"] = np.ascontiguousarray(
            w3[e].astype(bf16).reshape(MT, P, KT, P).transpose(0, 3, 2, 1)
        ).reshape(MT, P, KT * P)
        im["w2t"] = np.ascontiguousarray(w2[e].T.astype(bf16)).reshape(MT, P, DIM)
        in_maps.append(im)

    res = bass_utils.run_bass_kernel_spmd(
        nc, in_maps, core_ids=list(range(NCORES)), trace=TRACE
    )
    LAST_RESULTS = res

    out = np.empty((T * TOPK, DIM), dtype=np.float32)
    for e in range(E):
        slots = order[starts[e]:starts[e + 1]]
        yt = res.results[e]["yt"]  # [KT, P, C] f32
        y = yt.reshape(DIM, C)  # y.T
        out[slots] = y[:, : len(slots)].T
    return out.reshape(T, TOPK, DIM)
